# revision 18
# baseline (speedup 1.0000x reference)
"""BiMambaLM Trainium2 kernel: 8 NeuronCores, batch-grouped tensor-parallel.

Sharding: cores 0-3 compute batch 0, cores 4-7 batch 1. Within a 4-core
group each core owns 256 of the 1024 d_inner channels (both directions)
for in_proj/conv/scan/out_proj, plus 8000 of the 32000 vocab rows of the
tied lm_head for its batch. Per layer: one per-direction 4-core AllReduce
for the x_proj outputs (dt/B/C) and one for the out_proj partials.

v2: f16 matmuls, Silu/Sigmoid/Ln fused activations, sigmoid-power dA
(S4D structure), partition_broadcast for B/C replication, C-multiply on
the Pool engine, per-direction AllReduce software pipelining.
"""
import os
import sys

for _p in ("/opt/trn_rl_repo", "/opt/pypackages"):
    if os.path.isdir(_p) and _p not in sys.path:
        sys.path.append(_p)

import numpy as np

import concourse.bacc as bacc
import concourse.mybir as mybir
import concourse.tile as tile
from concourse.bass_utils import run_bass_kernel_spmd

F32 = mybir.dt.float32
F32R = mybir.dt.float32r
F16 = mybir.dt.float16
AF = mybir.ActivationFunctionType
OP = mybir.AluOpType

D = 512
N = 16
ED = 1024
DCONV = 4
DTR = 32
DEPTH = 6
VOCAB = 32000
B, L = 2, 512
EPS = 1e-5

N_CORES = 8
GROUP = 4            # cores per batch group
EC = ED // GROUP     # 256 channels per core per dir
NJ = EC // 128       # 2 partition tiles of 128 channels
VS = VOCAB // GROUP  # 8000 vocab rows per core
VSP = 8064           # padded to 63*128
NSEG = N * L         # 8192 free elements per scan tile
R2 = DTR + 2 * N     # 64 x_proj rows per dir
EGRP, ETIL = 21, 3   # lm_head: 21 groups of 3 m-tiles (63 * 128 = 8064)

_BUILT = {}


def _build(generic_exp: bool):
    nc = bacc.Bacc("TRN2", target_bir_lowering=False, debug=False,
                   num_devices=N_CORES)

    def din(name, shape, dtype=F32):
        return nc.dram_tensor(name, list(shape), dtype, kind="ExternalInput")

    x0_t = din("x0", [4, 128, L])
    winT_t = din("winT", [DEPTH, 128, 2, 4, 2 * EC], F16)
    convD_t = din("convD", [DEPTH, 2, 128, NJ, DCONV, 128], F16)
    cb_t = din("cb", [DEPTH, 2, 128, NJ])
    wxpT_t = din("wxpT", [DEPTH, 2, 128, NJ, R2], F16)
    wdtT_t = din("wdtT", [DEPTH, 2, DTR, NJ, 128], F16)
    bdt_t = din("bdt", [DEPTH, 2, 128, NJ])
    aexp_t = din("aexp", [DEPTH, 2, 128, NJ, N])
    dpD_t = din("dpD", [DEPTH, 2, 128, NJ, 128], F16)
    woutT_t = din("woutT", [DEPTH, 2, 128, NJ, 4, 128], F16)
    eT_t = din("eT", [EGRP, 4, 128, ETIL * 128], F16)
    ones1_t = din("ones1", [1, 128], F16)
    zero3_t = din("zero3", [128, 3], F16)
    onesc_t = din("onesc", [128, 1], F16)
    ident_t = din("ident", [128, 128], F16)

    logits_t = nc.dram_tensor("logits", [VSP, L], F16, kind="ExternalOutput")
    groups = [[0, 1, 2, 3], [4, 5, 6, 7]]

    with tile.TileContext(nc) as tc:
        with (
            tc.tile_pool(name="state", bufs=1) as stp,
            tc.tile_pool(name="winp", bufs=1) as winp,
            tc.tile_pool(name="wpool", bufs=2) as wp,
            tc.tile_pool(name="etp", bufs=2) as etp,
            tc.tile_pool(name="work", bufs=1) as kp,
            tc.tile_pool(name="big", bufs=1) as bigp,
            tc.tile_pool(name="ps", bufs=1, space="PSUM") as ps,
            tc.tile_pool(name="ps2", bufs=2, space="PSUM") as ps2,
            tc.tile_pool(name="dramp", bufs=2, space="DRAM") as dp,
        ):
            xst = [stp.tile([128, L], F32, tag=f"x{i}", name=f"x{i}")
                   for i in range(4)]
            for i in range(4):
                nc.sync.dma_start(xst[i][:], x0_t.ap()[i])
            ones1 = stp.tile([1, 128], F16, tag="ones1", name="ones1")
            nc.sync.dma_start(ones1[:], ones1_t.ap())
            onesc = stp.tile([128, 1], F16, tag="onesc", name="onesc")
            nc.sync.dma_start(onesc[:], onesc_t.ap())
            ident = stp.tile([128, 128], F16, tag="ident", name="ident")
            nc.sync.dma_start(ident[:], ident_t.ap())
            epsc = stp.tile([128, 1], F32, tag="epsc", name="epsc")
            nc.vector.memset(epsc[:], EPS)
            xev = {}
            for dd in range(2):
                for j in range(NJ):
                    xev[(dd, j)] = stp.tile([128, 3 + L], F16,
                                            tag=f"xev{dd}{j}",
                                            name=f"xev{dd}{j}")
                    pad = slice(0, 3) if dd == 0 else slice(L, L + 3)
                    nc.sync.dma_start(xev[(dd, j)][:, pad], zero3_t.ap())

            def rmsnorm_tiles(tag):
                sq = [kp.tile([128, L], F16, tag=f"sq{i % 2}",
                              name=f"sq{i}_{tag}") for i in range(4)]
                for i in range(4):
                    nc.vector.tensor_tensor(sq[i][:], xst[i][:], xst[i][:],
                                            OP.mult)
                sig = ps.tile([1, L], F32, tag="psR", name=f"sig_{tag}")
                for i in range(4):
                    nc.tensor.matmul(sig[:], onesc[:], sq[i][:],
                                     start=(i == 0), stop=(i == 3))
                lnm = kp.tile([1, L], F32, tag="lnm", name=f"lnm_{tag}")
                nc.scalar.activation(lnm[:], sig[:], AF.Ln,
                                     scale=1.0 / D, bias=epsc[0:1, :])
                rs = kp.tile([1, L], F16, tag="rs", name=f"rs_{tag}")
                nc.scalar.activation(rs[:], lnm[:], AF.Exp, scale=-0.5)
                rsp = ps.tile([128, L], F32, tag="psR", name=f"rsp_{tag}")
                nc.tensor.matmul(rsp[:], ones1[:], rs[:],
                                 start=True, stop=True)
                xn = [kp.tile([128, L], F16, tag=f"xn{i}",
                              name=f"xn{i}_{tag}") for i in range(4)]
                for i in range(4):
                    nc.vector.tensor_tensor(xn[i][:], xst[i][:],
                                            rsp[:], OP.mult)
                return xn

            for l in range(DEPTH):
                xn = rmsnorm_tiles(f"l{l}")

                winT = winp.tile([128, 2, 4, 2 * EC], F16, tag="winT",
                               name=f"winT{l}")
                nc.sync.dma_start(winT[:], winT_t.ap()[l])

                # ---- phase A: pre-AllReduce work, both dirs ----
                xsS, zS, bci, bco = {}, {}, {}, {}
                for d in range(2):
                    convD = winp.tile([128, NJ, DCONV, 128], F16,
                                    tag=f"convD{d}", name=f"convD{l}{d}")
                    nc.sync.dma_start(convD[:], convD_t.ap()[l, d])
                    cbw = wp.tile([128, NJ], F32, tag=f"cbw{d}",
                                  name=f"cbw{l}{d}")
                    nc.sync.dma_start(cbw[:], cb_t.ap()[l, d])
                    wxpT = wp.tile([128, NJ, R2], F16, tag=f"wxpT{d}",
                                   name=f"wxpT{l}{d}")
                    nc.sync.dma_start(wxpT[:], wxpT_t.ap()[l, d])

                    dblp = ps.tile([R2, L], F32, tag=f"dblp{d}",
                                   name=f"dblp{l}{d}")
                    for j in range(NJ):
                        pxs = ps2.tile([128, L], F32, tag="pio",
                                       name=f"pxs{l}{d}{j}")
                        for k in range(4):
                            nc.tensor.matmul(
                                pxs[:], winT[:, d, k, j * 128:(j + 1) * 128],
                                xn[k][:], start=(k == 0), stop=(k == 3))
                        xsl = slice(3, 3 + L) if d == 0 else slice(0, L)
                        nc.vector.tensor_scalar_mul(xev[(d, j)][:, xsl],
                                                    pxs[:], 1.0)

                        pz = ps2.tile([128, L], F32, tag="pio",
                                      name=f"pz{l}{d}{j}")
                        for k in range(4):
                            nc.tensor.matmul(
                                pz[:],
                                winT[:, d, k, EC + j * 128:EC + (j + 1) * 128],
                                xn[k][:], start=(k == 0), stop=(k == 3))
                        zS[(d, j)] = kp.tile([128, L], F16, tag=f"zS{d}{j}",
                                             name=f"zS{l}{d}{j}")
                        nc.scalar.activation(zS[(d, j)][:], pz[:], AF.Silu)

                        pcv = ps2.tile([128, L], F32, tag="pio",
                                       name=f"pcv{l}{d}{j}")
                        for k in range(DCONV):
                            off = k if d == 0 else 3 - k
                            nc.tensor.matmul(pcv[:], convD[:, j, k, :],
                                             xev[(d, j)][:, off:off + L],
                                             start=(k == 0),
                                             stop=(k == DCONV - 1))
                        xsS[(d, j)] = kp.tile([128, L], F16, tag=f"xsS{d}{j}",
                                              name=f"xsS{l}{d}{j}")
                        nc.scalar.activation(xsS[(d, j)][:], pcv[:], AF.Silu,
                                             bias=cbw[:, j:j + 1])
                        nc.tensor.matmul(dblp[:], wxpT[:, j, :],
                                         xsS[(d, j)][:], start=(j == 0),
                                         stop=(j == NJ - 1))
                    dbsb = kp.tile([R2, L], F16, tag=f"dbsb{d}",
                                   name=f"dbsb{l}{d}")
                    nc.scalar.activation(dbsb[:], dblp[:], AF.Copy)
                    bci[d] = dp.tile([R2, L], F16, tag=f"bci{d}",
                                     name=f"bci{l}{d}")
                    nc.scalar.dma_start(bci[d][:], dbsb[:])
                    bco[d] = dp.tile([R2, L], F16, tag=f"bco{d}",
                                     name=f"bco{l}{d}")
                    nc.gpsimd.collective_compute(
                        "AllReduce", OP.add, replica_groups=groups,
                        ins=[bci[d].opt()], outs=[bco[d].opt()])

                # ---- phase B: post-AllReduce prep, both dirs ----
                wdtT = wp.tile([DTR, 2, NJ, 128], F16, tag="wdtT",
                               name=f"wdtT{l}")
                nc.sync.dma_start(wdtT[:], wdtT_t.ap()[l].rearrange(
                    "d p j c -> p d j c"))
                bdt = wp.tile([128, 2, NJ], F32, tag="bdt", name=f"bdt{l}")
                nc.sync.dma_start(bdt[:], bdt_t.ap()[l].rearrange(
                    "d p j -> p d j"))
                dpD = wp.tile([128, 2, NJ, 128], F16, tag="dpD",
                              name=f"dpD{l}")
                nc.sync.dma_start(dpD[:], dpD_t.ap()[l].rearrange(
                    "d p j c -> p d j c"))
                aex = wp.tile([128, 2, NJ, N], F32, tag="aex",
                              name=f"aex{l}")
                nc.sync.dma_start(aex[:], aexp_t.ap()[l].rearrange(
                    "d p j n -> p d j n"))

                dbl, brep, crep, delta = {}, {}, {}, {}
                for d in range(2):
                    dbl[d] = kp.tile([DTR, L], F16, tag=f"dbl{d}",
                                     name=f"dbl{l}{d}")
                    nc.scalar.dma_start(dbl[d][:], bco[d][0:DTR, :])
                    brep[d] = bigp.tile([128, NSEG], F16, tag=f"brep{d}",
                                        name=f"brep{l}{d}")
                    for p in range(4):
                        nc.gpsimd.dma_start(
                            brep[d][p:p + 1, :].rearrange(
                                "p (a b) -> p a b", a=N),
                            bco[d][DTR:DTR + N, :])
                    for k in (4, 8, 16, 32, 64):
                        nc.gpsimd.dma_start(brep[d][k:2 * k, :],
                                            brep[d][0:k, :])
                    crep[d] = bigp.tile([128, NSEG], F16, tag=f"crep{d}",
                                        name=f"crep{l}{d}")
                    for p in range(4):
                        nc.gpsimd.dma_start(
                            crep[d][p:p + 1, :].rearrange(
                                "p (a b) -> p a b", a=N),
                            bco[d][DTR + N:R2, :])
                    for k in (4, 8, 16, 32, 64):
                        nc.gpsimd.dma_start(crep[d][k:2 * k, :],
                                            crep[d][0:k, :])

                    esp = {}
                    for j in range(NJ):
                        pdt = ps.tile([128, L], F32, tag="pdt",
                                      name=f"pdt{l}{d}{j}")
                        nc.tensor.matmul(pdt[:], wdtT[:, d, j, :],
                                         dbl[d][0:DTR, :],
                                         start=True, stop=True)
                        esp[j] = kp.tile([128, L], F32, tag=f"esp{j}",
                                         name=f"esp{l}{d}{j}")
                        nc.scalar.activation(esp[j][:], pdt[:], AF.Exp,
                                             bias=bdt[:, d, j:j + 1])
                    for j in range(NJ):
                        delta[(d, j)] = kp.tile([128, L], F16,
                                                tag=f"dlt{d}{j}",
                                                name=f"dlt{l}{d}{j}")
                        nc.scalar.activation(delta[(d, j)][:], esp[j][:],
                                             AF.Ln, bias=1.0)

                # ---- phase C: scan + gate, 1-tile software pipeline ----
                yg, oci, oco, dBxs = {}, {}, {}, {}

                dAs = {}

                def emit_exps(d, j):
                    dA = bigp.tile([128, NSEG], F16, tag=f"dA{j}",
                                   name=f"dA{l}{d}{j}")
                    dAs[(d, j)] = dA
                    nexps = N if generic_exp else 8
                    for n in range(nexps):
                        nc.scalar.activation(dA[:, n * L:(n + 1) * L],
                                             delta[(d, j)][:], AF.Exp,
                                             scale=aex[:, d, j, n:n + 1])

                def emit_prep_scan(d, j):
                    dA = dAs[(d, j)]
                    ubf = kp.tile([128, L], F16, tag=f"ubf{j}",
                                  name=f"ubf{l}{d}{j}")
                    nc.vector.tensor_tensor(ubf[:], delta[(d, j)][:],
                                            xsS[(d, j)][:], OP.mult)
                    if not generic_exp:
                        half = 8 * L
                        nc.vector.tensor_tensor(
                            dA[:, half:2 * half].rearrange(
                                "p (n t) -> p n t", n=8),
                            dA[:, 0:half].rearrange(
                                "p (n t) -> p n t", n=8),
                            dA[:, 7 * L:8 * L].unsqueeze(1)
                            .broadcast_to([128, 8, L]),
                            OP.mult)
                    dBx = bigp.tile([128, NSEG], F16, tag=f"dBx{j}",
                                    name=f"dBx{l}{d}{j}")
                    dBxs[(d, j)] = dBx
                    nc.vector.tensor_tensor(
                        dBx[:].rearrange("p (n t) -> p n t", n=N),
                        ubf[:].unsqueeze(1).broadcast_to([128, N, L]),
                        brep[d][:].rearrange("p (n t) -> p n t", n=N),
                        OP.mult)
                    rcol = slice(0, 1) if d == 0 else slice(L - 1, L)
                    nc.vector.memset(
                        dA[:].rearrange("p (n t) -> p n t",
                                        n=N)[:, :, rcol], 0.0)
                    if d == 0:
                        nc.vector.tensor_tensor_scan(
                            dBx[:], dA[:], dBx[:], 0.0, OP.mult, OP.add)
                    else:
                        nc.vector.tensor_tensor_scan(
                            dBx[:, ::-1], dA[:, ::-1], dBx[:, ::-1],
                            0.0, OP.mult, OP.add)
                    nc.vector.tensor_tensor(dBx[:], dBx[:], crep[d][:],
                                            OP.mult)

                def emit_reduce_mm(d, j):
                    # n-state reduction via ident matmuls into PSUM
                    dBx = dBxs[(d, j)]
                    py = ps2.tile([128, L], F32, tag="py",
                                  name=f"py{l}{d}{j}")
                    for n in range(N):
                        nc.tensor.matmul(py[:], ident[:],
                                         dBx[:, n * L:(n + 1) * L],
                                         start=(n == 0), stop=False)
                    nc.tensor.matmul(py[:], dpD[:, d, j, :],
                                     xsS[(d, j)][:],
                                     start=False, stop=True)
                    return py

                pys = {}

                def emit_yg(d, j):
                    yg[(d, j)] = kp.tile([128, L], F16, tag=f"yg{d}{j}",
                                         name=f"yg{l}{d}{j}")
                    nc.vector.tensor_tensor(yg[(d, j)][:],
                                            pys[(d, j)][:], zS[(d, j)][:],
                                            OP.mult)

                def emit_yg_tree(d, j):
                    # DVE add-tree over n + dpD matmul (layer tail: keep
                    # the reduction off the tensor queue)
                    dBx = dBxs[(d, j)]
                    for w in (8, 4, 2, 1):
                        nc.vector.tensor_tensor(
                            dBx[:, 0:w * L], dBx[:, 0:w * L],
                            dBx[:, w * L:2 * w * L], OP.add)
                    py = ps2.tile([128, L], F32, tag="py",
                                  name=f"py{l}{d}{j}")
                    nc.tensor.matmul(py[:], dpD[:, d, j, :],
                                     xsS[(d, j)][:], start=True, stop=True)
                    ytmp = kp.tile([128, L], F16, tag="ytmp",
                                   name=f"ytmp{l}{d}{j}")
                    nc.vector.tensor_tensor(ytmp[:], py[:], dBx[:, 0:L],
                                            OP.add)
                    yg[(d, j)] = kp.tile([128, L], F16, tag=f"yg{d}{j}",
                                         name=f"yg{l}{d}{j}")
                    nc.vector.tensor_tensor(yg[(d, j)][:],
                                            ytmp[:], zS[(d, j)][:], OP.mult)

                def emit_out(d):
                    woutT = winp.tile([128, NJ, 4, 128], F16,
                                      tag=f"woutT{d}", name=f"woutT{l}{d}")
                    nc.sync.dma_start(woutT[:], woutT_t.ap()[l, d])
                    oci[d] = dp.tile([D, L], F16, tag=f"oci{d}",
                                     name=f"oci{l}{d}")
                    for g in range(4):
                        pog = ps2.tile([128, L], F32, tag="pio",
                                       name=f"pout{l}{d}{g}")
                        for j in range(NJ):
                            nc.tensor.matmul(pog[:], woutT[:, j, g, :],
                                             yg[(d, j)][:], start=(j == 0),
                                             stop=(j == NJ - 1))
                        posb = kp.tile([128, L], F16, tag="posb",
                                       name=f"posb{l}{d}{g}")
                        if d == 0:
                            nc.scalar.activation(posb[:], pog[:], AF.Copy)
                        else:
                            nc.vector.tensor_scalar_mul(posb[:], pog[:], 1.0)
                        nc.scalar.dma_start(oci[d][g * 128:(g + 1) * 128, :],
                                            posb[:])
                    oco[d] = dp.tile([D, L], F16, tag=f"oco{d}",
                                     name=f"oco{l}{d}")
                    nc.gpsimd.collective_compute(
                        "AllReduce", OP.add, replica_groups=groups,
                        ins=[oci[d].opt()], outs=[oco[d].opt()])

                for dd in range(2):
                    for jj in range(NJ):
                        emit_exps(dd, jj)
                emit_prep_scan(0, 0)
                pys[(0, 0)] = emit_reduce_mm(0, 0)
                emit_prep_scan(0, 1)
                pys[(0, 1)] = emit_reduce_mm(0, 1)
                emit_yg(0, 0)
                emit_prep_scan(1, 0)
                pys[(1, 0)] = emit_reduce_mm(1, 0)
                emit_yg(0, 1)
                emit_out(0)
                emit_prep_scan(1, 1)
                emit_yg(1, 0)
                emit_yg_tree(1, 1)
                emit_out(1)

                # ---- phase D: residual adds, out0 first then out1 ----
                for dd in range(2):
                    for i in range(4):
                        xadd = kp.tile([128, L], F16, tag=f"xadd{i % 2}",
                                       name=f"xadd{l}{i}{dd}")
                        nc.gpsimd.dma_start(
                            xadd[:], oco[dd][i * 128:(i + 1) * 128, :])
                        nc.vector.tensor_tensor(xst[i][:], xst[i][:],
                                                xadd[:], OP.add)

            xf = rmsnorm_tiles("fin")
            for gi in range(EGRP):
                eT = etp.tile([128, 4, ETIL * 128], F16, tag="eT",
                              name=f"eT{gi}")
                nc.sync.dma_start(eT[:], eT_t.ap()[gi].rearrange(
                    "k p c -> p k c"))
                for mt in range(ETIL):
                    m = gi * ETIL + mt
                    plm = ps2.tile([128, L], F32,
                                   tag="pio" if m % 2 else "py",
                                   name=f"plm{m}")
                    for k in range(4):
                        nc.tensor.matmul(
                            plm[:], eT[:, k, mt * 128:(mt + 1) * 128],
                            xf[k][:], start=(k == 0), stop=(k == 3))
                    lmsb = kp.tile([128, L], F16, tag="posb",
                                   name=f"lmsb{m}")
                    nc.scalar.activation(lmsb[:], plm[:], AF.Copy)
                    nc.scalar.dma_start(
                        logits_t.ap()[m * 128:(m + 1) * 128, :], lmsb[:])

    nc.compile()
    return nc


def _prep_inputs(inputs):
    tokens = np.asarray(inputs["tokens"])
    E = np.asarray(inputs["E"], np.float32)
    norm_w = np.asarray(inputs["norm_w"], np.float32)
    W_in = np.asarray(inputs["W_in"], np.float32)
    conv_w = np.asarray(inputs["conv_w"], np.float32)
    conv_b = np.asarray(inputs["conv_b"], np.float32)
    W_xp = np.asarray(inputs["W_xp"], np.float32)
    W_dt = np.asarray(inputs["W_dt"], np.float32)
    b_dt = np.asarray(inputs["b_dt"], np.float32)
    A_log = np.asarray(inputs["A_log"], np.float32)
    Dparam = np.asarray(inputs["Dparam"], np.float32)
    W_out = np.asarray(inputs["W_out"], np.float32)
    out_norm_w = np.asarray(inputs["out_norm_w"], np.float32)

    A = -np.exp(A_log)  # [DEPTH, 2, ED, N]
    struct_ok = bool(np.allclose(A[..., 8:16], A[..., 7:8] + A[..., 0:8],
                                 rtol=1e-6, atol=1e-7))

    f16 = np.float16
    in_maps = []
    for c in range(N_CORES):
        g, r = divmod(c, GROUP)
        e0 = r * EC
        m = {}
        m["x0"] = np.ascontiguousarray(
            E[tokens[g]].T.astype(np.float32).reshape(4, 128, L))

        winT = np.empty((DEPTH, 128, 2, 4, 2 * EC), f16)
        convD = np.zeros((DEPTH, 2, 128, NJ, DCONV, 128), f16)
        cb = np.empty((DEPTH, 2, 128, NJ), np.float32)
        wxpT = np.empty((DEPTH, 2, 128, NJ, R2), f16)
        wdtT = np.empty((DEPTH, 2, DTR, NJ, 128), f16)
        bdt = np.empty((DEPTH, 2, 128, NJ), np.float32)
        aexp = np.empty((DEPTH, 2, 128, NJ, N), np.float32)
        dpD = np.zeros((DEPTH, 2, 128, NJ, 128), f16)
        woutT = np.empty((DEPTH, 2, 128, NJ, 4, 128), f16)
        idx = np.arange(128)
        for l in range(DEPTH):
            for d in range(2):
                Wf = W_in[l, d] * norm_w[l][None, :]
                rows = np.concatenate([Wf[e0:e0 + EC, :],
                                       Wf[ED + e0:ED + e0 + EC, :]], 0)
                winT[l, :, d] = rows.T.reshape(4, 128, 2 * EC).transpose(
                    1, 0, 2).astype(f16)
                Wxp = W_xp[l, d]
                for j in range(NJ):
                    ej = slice(e0 + j * 128, e0 + (j + 1) * 128)
                    for k in range(DCONV):
                        convD[l, d, idx, j, k, idx] = \
                            conv_w[l, d, ej, k].astype(f16)
                    cb[l, d, :, j] = conv_b[l, d, ej]
                    wxpT[l, d, :, j, :] = Wxp[:, ej].T.astype(f16)
                    wdtT[l, d, :, j, :] = W_dt[l, d][ej, :].T.astype(f16)
                    bdt[l, d, :, j] = b_dt[l, d, ej]
                    aexp[l, d, :, j, :] = A[l, d, ej, :]
                    dpD[l, d, idx, j, idx] = Dparam[l, d, ej].astype(f16)
                    for gg in range(4):
                        woutT[l, d, :, j, gg, :] = \
                            W_out[l, d][gg * 128:(gg + 1) * 128, ej].T \
                            .astype(f16)
        m["winT"] = winT
        m["convD"] = convD
        m["cb"] = cb
        m["wxpT"] = wxpT
        m["wdtT"] = wdtT
        m["bdt"] = bdt
        m["aexp"] = aexp
        m["dpD"] = dpD
        m["woutT"] = woutT

        Ev = np.zeros((VSP, D), np.float32)
        Ev[:VS] = E[r * VS:(r + 1) * VS] * out_norm_w[None, :]
        m["eT"] = np.ascontiguousarray(
            Ev.T.reshape(4, 128, EGRP, ETIL * 128).transpose(
                2, 0, 1, 3)).astype(f16)
        m["ones1"] = np.ones((1, 128), f16)
        m["zero3"] = np.zeros((128, 3), f16)
        m["onesc"] = np.ones((128, 1), f16)
        m["ident"] = np.eye(128).astype(f16)
        in_maps.append(m)
    return in_maps, struct_ok


def kernel(**inputs):
    in_maps, struct_ok = _prep_inputs(inputs)
    key = not struct_ok
    if key not in _BUILT:
        _BUILT[key] = _build(generic_exp=key)
    nc = _BUILT[key]
    res = run_bass_kernel_spmd(nc, in_maps, core_ids=list(range(N_CORES)))
    out = np.empty((B, L, VOCAB), np.float32)
    for c in range(N_CORES):
        g, r = divmod(c, GROUP)
        out[g, :, r * VS:(r + 1) * VS] = \
            res.results[c]["logits"][:VS].T.astype(np.float32)
    return out


if __name__ == "__main__":
    sys.path.insert(0, os.path.dirname(os.path.abspath(__file__)))
    import reference
    ins = {k: np.asarray(v) for k, v in reference.setup_inputs().items()}
    got = kernel(**ins)
    exp = np.asarray(reference.reference(**ins))
    rel = np.abs(got - exp).max() / np.abs(exp).max()
    print("Relative error:", rel)


# revision 19
# speedup vs baseline: 1.0119x; 1.0119x over previous
"""BiMambaLM Trainium2 kernel: 8 NeuronCores, batch-grouped tensor-parallel.

Sharding: cores 0-3 compute batch 0, cores 4-7 batch 1. Within a 4-core
group each core owns 256 of the 1024 d_inner channels (both directions)
for in_proj/conv/scan/out_proj, plus 8000 of the 32000 vocab rows of the
tied lm_head for its batch. Per layer: one per-direction 4-core AllReduce
for the x_proj outputs (dt/B/C) and one for the out_proj partials.

v2: f16 matmuls, Silu/Sigmoid/Ln fused activations, sigmoid-power dA
(S4D structure), partition_broadcast for B/C replication, C-multiply on
the Pool engine, per-direction AllReduce software pipelining.
"""
import os
import sys

for _p in ("/opt/trn_rl_repo", "/opt/pypackages"):
    if os.path.isdir(_p) and _p not in sys.path:
        sys.path.append(_p)

import numpy as np

import concourse.bacc as bacc
import concourse.mybir as mybir
import concourse.tile as tile
from concourse.bass_utils import run_bass_kernel_spmd

F32 = mybir.dt.float32
F32R = mybir.dt.float32r
F16 = mybir.dt.float16
AF = mybir.ActivationFunctionType
OP = mybir.AluOpType

D = 512
N = 16
ED = 1024
DCONV = 4
DTR = 32
DEPTH = 6
VOCAB = 32000
B, L = 2, 512
EPS = 1e-5

N_CORES = 8
GROUP = 4            # cores per batch group
EC = ED // GROUP     # 256 channels per core per dir
NJ = EC // 128       # 2 partition tiles of 128 channels
VS = VOCAB // GROUP  # 8000 vocab rows per core
VSP = 8064           # padded to 63*128
NSEG = N * L         # 8192 free elements per scan tile
R2 = DTR + 2 * N     # 64 x_proj rows per dir
EGRP, ETIL = 21, 3   # lm_head: 21 groups of 3 m-tiles (63 * 128 = 8064)

_BUILT = {}


def _build(generic_exp: bool):
    nc = bacc.Bacc("TRN2", target_bir_lowering=False, debug=False,
                   num_devices=N_CORES)

    def din(name, shape, dtype=F32):
        return nc.dram_tensor(name, list(shape), dtype, kind="ExternalInput")

    x0_t = din("x0", [4, 128, L])
    winT_t = din("winT", [DEPTH, 128, 2, 4, 2 * EC], F16)
    convD_t = din("convD", [DEPTH, 2, 128, NJ, DCONV, 128], F16)
    cb_t = din("cb", [DEPTH, 2, 128, NJ])
    wxpT_t = din("wxpT", [DEPTH, 2, 128, NJ, R2], F16)
    wdtT_t = din("wdtT", [DEPTH, 2, DTR, NJ, 128], F16)
    bdt_t = din("bdt", [DEPTH, 2, 128, NJ])
    aexp_t = din("aexp", [DEPTH, 2, 128, NJ, N])
    dpD_t = din("dpD", [DEPTH, 2, 128, NJ, 128], F16)
    woutT_t = din("woutT", [DEPTH, 2, 128, NJ, 4, 128], F16)
    eT_t = din("eT", [EGRP, 4, 128, ETIL * 128], F16)
    ones1_t = din("ones1", [1, 128], F16)
    zero3_t = din("zero3", [128, 3], F16)
    onesc_t = din("onesc", [128, 1], F16)
    ident_t = din("ident", [128, 128], F16)

    logits_t = nc.dram_tensor("logits", [VSP, L], F16, kind="ExternalOutput")
    groups = [[0, 1, 2, 3], [4, 5, 6, 7]]

    with tile.TileContext(nc) as tc:
        with (
            tc.tile_pool(name="state", bufs=1) as stp,
            tc.tile_pool(name="winp", bufs=1) as winp,
            tc.tile_pool(name="wpool", bufs=2) as wp,
            tc.tile_pool(name="etp", bufs=2) as etp,
            tc.tile_pool(name="work", bufs=1) as kp,
            tc.tile_pool(name="big", bufs=1) as bigp,
            tc.tile_pool(name="ps", bufs=1, space="PSUM") as ps,
            tc.tile_pool(name="ps2", bufs=2, space="PSUM") as ps2,
            tc.tile_pool(name="dramp", bufs=2, space="DRAM") as dp,
        ):
            xst = [stp.tile([128, L], F32, tag=f"x{i}", name=f"x{i}")
                   for i in range(4)]
            for i in range(4):
                nc.sync.dma_start(xst[i][:], x0_t.ap()[i])
            ones1 = stp.tile([1, 128], F16, tag="ones1", name="ones1")
            nc.sync.dma_start(ones1[:], ones1_t.ap())
            onesc = stp.tile([128, 1], F16, tag="onesc", name="onesc")
            nc.sync.dma_start(onesc[:], onesc_t.ap())
            ident = stp.tile([128, 128], F16, tag="ident", name="ident")
            nc.sync.dma_start(ident[:], ident_t.ap())
            epsc = stp.tile([128, 1], F32, tag="epsc", name="epsc")
            nc.vector.memset(epsc[:], EPS)
            xev = {}
            for dd in range(2):
                for j in range(NJ):
                    xev[(dd, j)] = stp.tile([128, 3 + L], F16,
                                            tag=f"xev{dd}{j}",
                                            name=f"xev{dd}{j}")
                    pad = slice(0, 3) if dd == 0 else slice(L, L + 3)
                    nc.sync.dma_start(xev[(dd, j)][:, pad], zero3_t.ap())

            def rmsnorm_tiles(tag):
                sq = [kp.tile([128, L], F16, tag=f"sq{i % 2}",
                              name=f"sq{i}_{tag}") for i in range(4)]
                for i in range(4):
                    nc.scalar.activation(sq[i][:], xst[i][:], AF.Square)
                sig = ps.tile([1, L], F32, tag="psR", name=f"sig_{tag}")
                for i in range(4):
                    nc.tensor.matmul(sig[:], onesc[:], sq[i][:],
                                     start=(i == 0), stop=(i == 3))
                lnm = kp.tile([1, L], F32, tag="lnm", name=f"lnm_{tag}")
                nc.scalar.activation(lnm[:], sig[:], AF.Ln,
                                     scale=1.0 / D, bias=epsc[0:1, :])
                rs = kp.tile([1, L], F16, tag="rs", name=f"rs_{tag}")
                nc.scalar.activation(rs[:], lnm[:], AF.Exp, scale=-0.5)
                rsp = ps.tile([128, L], F32, tag="psR", name=f"rsp_{tag}")
                nc.tensor.matmul(rsp[:], ones1[:], rs[:],
                                 start=True, stop=True)
                xn = [kp.tile([128, L], F16, tag=f"xn{i}",
                              name=f"xn{i}_{tag}") for i in range(4)]
                for i in range(4):
                    nc.vector.tensor_tensor(xn[i][:], xst[i][:],
                                            rsp[:], OP.mult)
                return xn

            for l in range(DEPTH):
                xn = rmsnorm_tiles(f"l{l}")

                winT = winp.tile([128, 2, 4, 2 * EC], F16, tag="winT",
                               name=f"winT{l}")
                nc.sync.dma_start(winT[:], winT_t.ap()[l])

                # ---- phase A: pre-AllReduce work, both dirs ----
                xsS, zS, bci, bco = {}, {}, {}, {}
                for d in range(2):
                    convD = winp.tile([128, NJ, DCONV, 128], F16,
                                    tag=f"convD{d}", name=f"convD{l}{d}")
                    nc.sync.dma_start(convD[:], convD_t.ap()[l, d])
                    cbw = wp.tile([128, NJ], F32, tag=f"cbw{d}",
                                  name=f"cbw{l}{d}")
                    nc.sync.dma_start(cbw[:], cb_t.ap()[l, d])
                    wxpT = wp.tile([128, NJ, R2], F16, tag=f"wxpT{d}",
                                   name=f"wxpT{l}{d}")
                    nc.sync.dma_start(wxpT[:], wxpT_t.ap()[l, d])

                    dblp = ps.tile([R2, L], F32, tag=f"dblp{d}",
                                   name=f"dblp{l}{d}")
                    for j in range(NJ):
                        pxs = ps2.tile([128, L], F32, tag="pio",
                                       name=f"pxs{l}{d}{j}")
                        for k in range(4):
                            nc.tensor.matmul(
                                pxs[:], winT[:, d, k, j * 128:(j + 1) * 128],
                                xn[k][:], start=(k == 0), stop=(k == 3))
                        xsl = slice(3, 3 + L) if d == 0 else slice(0, L)
                        nc.scalar.activation(xev[(d, j)][:, xsl], pxs[:],
                                             AF.Copy)

                        pz = ps2.tile([128, L], F32, tag="pio",
                                      name=f"pz{l}{d}{j}")
                        for k in range(4):
                            nc.tensor.matmul(
                                pz[:],
                                winT[:, d, k, EC + j * 128:EC + (j + 1) * 128],
                                xn[k][:], start=(k == 0), stop=(k == 3))
                        zS[(d, j)] = kp.tile([128, L], F16, tag=f"zS{d}{j}",
                                             name=f"zS{l}{d}{j}")
                        nc.scalar.activation(zS[(d, j)][:], pz[:], AF.Silu)

                        pcv = ps2.tile([128, L], F32, tag="pio",
                                       name=f"pcv{l}{d}{j}")
                        for k in range(DCONV):
                            off = k if d == 0 else 3 - k
                            nc.tensor.matmul(pcv[:], convD[:, j, k, :],
                                             xev[(d, j)][:, off:off + L],
                                             start=(k == 0),
                                             stop=(k == DCONV - 1))
                        xsS[(d, j)] = kp.tile([128, L], F16, tag=f"xsS{d}{j}",
                                              name=f"xsS{l}{d}{j}")
                        nc.scalar.activation(xsS[(d, j)][:], pcv[:], AF.Silu,
                                             bias=cbw[:, j:j + 1])
                        nc.tensor.matmul(dblp[:], wxpT[:, j, :],
                                         xsS[(d, j)][:], start=(j == 0),
                                         stop=(j == NJ - 1))
                    dbsb = kp.tile([R2, L], F16, tag=f"dbsb{d}",
                                   name=f"dbsb{l}{d}")
                    nc.scalar.activation(dbsb[:], dblp[:], AF.Copy)
                    bci[d] = dp.tile([R2, L], F16, tag=f"bci{d}",
                                     name=f"bci{l}{d}")
                    nc.scalar.dma_start(bci[d][:], dbsb[:])
                    bco[d] = dp.tile([R2, L], F16, tag=f"bco{d}",
                                     name=f"bco{l}{d}")
                    nc.gpsimd.collective_compute(
                        "AllReduce", OP.add, replica_groups=groups,
                        ins=[bci[d].opt()], outs=[bco[d].opt()])

                # ---- phase B: post-AllReduce prep, both dirs ----
                wdtT = wp.tile([DTR, 2, NJ, 128], F16, tag="wdtT",
                               name=f"wdtT{l}")
                nc.sync.dma_start(wdtT[:], wdtT_t.ap()[l].rearrange(
                    "d p j c -> p d j c"))
                bdt = wp.tile([128, 2, NJ], F32, tag="bdt", name=f"bdt{l}")
                nc.sync.dma_start(bdt[:], bdt_t.ap()[l].rearrange(
                    "d p j -> p d j"))
                dpD = wp.tile([128, 2, NJ, 128], F16, tag="dpD",
                              name=f"dpD{l}")
                nc.sync.dma_start(dpD[:], dpD_t.ap()[l].rearrange(
                    "d p j c -> p d j c"))
                aex = wp.tile([128, 2, NJ, N], F32, tag="aex",
                              name=f"aex{l}")
                nc.sync.dma_start(aex[:], aexp_t.ap()[l].rearrange(
                    "d p j n -> p d j n"))

                dbl, brep, crep, delta = {}, {}, {}, {}
                for d in range(2):
                    dbl[d] = kp.tile([DTR, L], F16, tag=f"dbl{d}",
                                     name=f"dbl{l}{d}")
                    nc.scalar.dma_start(dbl[d][:], bco[d][0:DTR, :])
                    brep[d] = bigp.tile([128, NSEG], F16, tag=f"brep{d}",
                                        name=f"brep{l}{d}")
                    for p in range(4):
                        nc.gpsimd.dma_start(
                            brep[d][p:p + 1, :].rearrange(
                                "p (a b) -> p a b", a=N),
                            bco[d][DTR:DTR + N, :])
                    for k in (4, 8, 16, 32, 64):
                        nc.gpsimd.dma_start(brep[d][k:2 * k, :],
                                            brep[d][0:k, :])
                    crep[d] = bigp.tile([128, NSEG], F16, tag=f"crep{d}",
                                        name=f"crep{l}{d}")
                    for p in range(4):
                        nc.gpsimd.dma_start(
                            crep[d][p:p + 1, :].rearrange(
                                "p (a b) -> p a b", a=N),
                            bco[d][DTR + N:R2, :])
                    for k in (4, 8, 16, 32, 64):
                        nc.gpsimd.dma_start(crep[d][k:2 * k, :],
                                            crep[d][0:k, :])

                    esp = {}
                    for j in range(NJ):
                        pdt = ps.tile([128, L], F32, tag="pdt",
                                      name=f"pdt{l}{d}{j}")
                        nc.tensor.matmul(pdt[:], wdtT[:, d, j, :],
                                         dbl[d][0:DTR, :],
                                         start=True, stop=True)
                        esp[j] = kp.tile([128, L], F32, tag=f"esp{j}",
                                         name=f"esp{l}{d}{j}")
                        nc.scalar.activation(esp[j][:], pdt[:], AF.Exp,
                                             bias=bdt[:, d, j:j + 1])
                    for j in range(NJ):
                        delta[(d, j)] = kp.tile([128, L], F16,
                                                tag=f"dlt{d}{j}",
                                                name=f"dlt{l}{d}{j}")
                        nc.scalar.activation(delta[(d, j)][:], esp[j][:],
                                             AF.Ln, bias=1.0)

                # ---- phase C: scan + gate, 1-tile software pipeline ----
                yg, oci, oco, dBxs = {}, {}, {}, {}

                dAs = {}

                def emit_exps(d, j):
                    dA = bigp.tile([128, NSEG], F16, tag=f"dA{j}",
                                   name=f"dA{l}{d}{j}")
                    dAs[(d, j)] = dA
                    nexps = N if generic_exp else 8
                    for n in range(nexps):
                        nc.scalar.activation(dA[:, n * L:(n + 1) * L],
                                             delta[(d, j)][:], AF.Exp,
                                             scale=aex[:, d, j, n:n + 1])

                def emit_prep_scan(d, j):
                    dA = dAs[(d, j)]
                    ubf = kp.tile([128, L], F16, tag=f"ubf{j}",
                                  name=f"ubf{l}{d}{j}")
                    nc.vector.tensor_tensor(ubf[:], delta[(d, j)][:],
                                            xsS[(d, j)][:], OP.mult)
                    if not generic_exp:
                        half = 8 * L
                        nc.vector.tensor_tensor(
                            dA[:, half:2 * half].rearrange(
                                "p (n t) -> p n t", n=8),
                            dA[:, 0:half].rearrange(
                                "p (n t) -> p n t", n=8),
                            dA[:, 7 * L:8 * L].unsqueeze(1)
                            .broadcast_to([128, 8, L]),
                            OP.mult)
                    dBx = bigp.tile([128, NSEG], F16, tag=f"dBx{j}",
                                    name=f"dBx{l}{d}{j}")
                    dBxs[(d, j)] = dBx
                    nc.vector.tensor_tensor(
                        dBx[:].rearrange("p (n t) -> p n t", n=N),
                        ubf[:].unsqueeze(1).broadcast_to([128, N, L]),
                        brep[d][:].rearrange("p (n t) -> p n t", n=N),
                        OP.mult)
                    rcol = slice(0, 1) if d == 0 else slice(L - 1, L)
                    nc.vector.memset(
                        dA[:].rearrange("p (n t) -> p n t",
                                        n=N)[:, :, rcol], 0.0)
                    if d == 0:
                        nc.vector.tensor_tensor_scan(
                            dBx[:], dA[:], dBx[:], 0.0, OP.mult, OP.add)
                    else:
                        nc.vector.tensor_tensor_scan(
                            dBx[:, ::-1], dA[:, ::-1], dBx[:, ::-1],
                            0.0, OP.mult, OP.add)
                    nc.vector.tensor_tensor(dBx[:], dBx[:], crep[d][:],
                                            OP.mult)

                def emit_reduce_mm(d, j):
                    # n-state reduction via ident matmuls into PSUM
                    dBx = dBxs[(d, j)]
                    py = ps2.tile([128, L], F32, tag="py",
                                  name=f"py{l}{d}{j}")
                    for n in range(N):
                        nc.tensor.matmul(py[:], ident[:],
                                         dBx[:, n * L:(n + 1) * L],
                                         start=(n == 0), stop=False)
                    nc.tensor.matmul(py[:], dpD[:, d, j, :],
                                     xsS[(d, j)][:],
                                     start=False, stop=True)
                    return py

                pys = {}

                def emit_yg(d, j):
                    yg[(d, j)] = kp.tile([128, L], F16, tag=f"yg{d}{j}",
                                         name=f"yg{l}{d}{j}")
                    nc.vector.tensor_tensor(yg[(d, j)][:],
                                            pys[(d, j)][:], zS[(d, j)][:],
                                            OP.mult)

                def emit_yg_tree(d, j):
                    # DVE add-tree over n + dpD matmul (layer tail: keep
                    # the reduction off the tensor queue)
                    dBx = dBxs[(d, j)]
                    for w in (8, 4, 2, 1):
                        nc.vector.tensor_tensor(
                            dBx[:, 0:w * L], dBx[:, 0:w * L],
                            dBx[:, w * L:2 * w * L], OP.add)
                    py = ps2.tile([128, L], F32, tag="py",
                                  name=f"py{l}{d}{j}")
                    nc.tensor.matmul(py[:], dpD[:, d, j, :],
                                     xsS[(d, j)][:], start=True, stop=True)
                    ytmp = kp.tile([128, L], F16, tag="ytmp",
                                   name=f"ytmp{l}{d}{j}")
                    nc.vector.tensor_tensor(ytmp[:], py[:], dBx[:, 0:L],
                                            OP.add)
                    yg[(d, j)] = kp.tile([128, L], F16, tag=f"yg{d}{j}",
                                         name=f"yg{l}{d}{j}")
                    nc.vector.tensor_tensor(yg[(d, j)][:],
                                            ytmp[:], zS[(d, j)][:], OP.mult)

                def emit_out(d):
                    woutT = winp.tile([128, NJ, 4, 128], F16,
                                      tag=f"woutT{d}", name=f"woutT{l}{d}")
                    nc.sync.dma_start(woutT[:], woutT_t.ap()[l, d])
                    oci[d] = dp.tile([D, L], F16, tag=f"oci{d}",
                                     name=f"oci{l}{d}")
                    for g in range(4):
                        pog = ps2.tile([128, L], F32, tag="pio",
                                       name=f"pout{l}{d}{g}")
                        for j in range(NJ):
                            nc.tensor.matmul(pog[:], woutT[:, j, g, :],
                                             yg[(d, j)][:], start=(j == 0),
                                             stop=(j == NJ - 1))
                        posb = kp.tile([128, L], F16, tag="posb",
                                       name=f"posb{l}{d}{g}")
                        nc.scalar.activation(posb[:], pog[:], AF.Copy)
                        nc.scalar.dma_start(oci[d][g * 128:(g + 1) * 128, :],
                                            posb[:])
                    oco[d] = dp.tile([D, L], F16, tag=f"oco{d}",
                                     name=f"oco{l}{d}")
                    nc.gpsimd.collective_compute(
                        "AllReduce", OP.add, replica_groups=groups,
                        ins=[oci[d].opt()], outs=[oco[d].opt()])

                for dd in range(2):
                    for jj in range(NJ):
                        emit_exps(dd, jj)
                emit_prep_scan(0, 0)
                pys[(0, 0)] = emit_reduce_mm(0, 0)
                emit_prep_scan(0, 1)
                pys[(0, 1)] = emit_reduce_mm(0, 1)
                emit_yg(0, 0)
                emit_prep_scan(1, 0)
                pys[(1, 0)] = emit_reduce_mm(1, 0)
                emit_yg(0, 1)
                emit_out(0)
                emit_prep_scan(1, 1)
                emit_yg(1, 0)
                emit_yg_tree(1, 1)
                emit_out(1)

                # ---- phase D: residual adds, out0 first then out1 ----
                for dd in range(2):
                    for i in range(4):
                        xadd = kp.tile([128, L], F16, tag=f"xadd{i % 2}",
                                       name=f"xadd{l}{i}{dd}")
                        nc.gpsimd.dma_start(
                            xadd[:], oco[dd][i * 128:(i + 1) * 128, :])
                        nc.vector.tensor_tensor(xst[i][:], xst[i][:],
                                                xadd[:], OP.add)

            xf = rmsnorm_tiles("fin")
            for gi in range(EGRP):
                eT = etp.tile([128, 4, ETIL * 128], F16, tag="eT",
                              name=f"eT{gi}")
                nc.sync.dma_start(eT[:], eT_t.ap()[gi].rearrange(
                    "k p c -> p k c"))
                for mt in range(ETIL):
                    m = gi * ETIL + mt
                    plm = ps2.tile([128, L], F32,
                                   tag="pio" if m % 2 else "py",
                                   name=f"plm{m}")
                    for k in range(4):
                        nc.tensor.matmul(
                            plm[:], eT[:, k, mt * 128:(mt + 1) * 128],
                            xf[k][:], start=(k == 0), stop=(k == 3))
                    lmsb = kp.tile([128, L], F16, tag="posb",
                                   name=f"lmsb{m}")
                    nc.scalar.activation(lmsb[:], plm[:], AF.Copy)
                    nc.scalar.dma_start(
                        logits_t.ap()[m * 128:(m + 1) * 128, :], lmsb[:])

    nc.compile()
    return nc


def _prep_inputs(inputs):
    tokens = np.asarray(inputs["tokens"])
    E = np.asarray(inputs["E"], np.float32)
    norm_w = np.asarray(inputs["norm_w"], np.float32)
    W_in = np.asarray(inputs["W_in"], np.float32)
    conv_w = np.asarray(inputs["conv_w"], np.float32)
    conv_b = np.asarray(inputs["conv_b"], np.float32)
    W_xp = np.asarray(inputs["W_xp"], np.float32)
    W_dt = np.asarray(inputs["W_dt"], np.float32)
    b_dt = np.asarray(inputs["b_dt"], np.float32)
    A_log = np.asarray(inputs["A_log"], np.float32)
    Dparam = np.asarray(inputs["Dparam"], np.float32)
    W_out = np.asarray(inputs["W_out"], np.float32)
    out_norm_w = np.asarray(inputs["out_norm_w"], np.float32)

    A = -np.exp(A_log)  # [DEPTH, 2, ED, N]
    struct_ok = bool(np.allclose(A[..., 8:16], A[..., 7:8] + A[..., 0:8],
                                 rtol=1e-6, atol=1e-7))

    f16 = np.float16
    in_maps = []
    for c in range(N_CORES):
        g, r = divmod(c, GROUP)
        e0 = r * EC
        m = {}
        m["x0"] = np.ascontiguousarray(
            E[tokens[g]].T.astype(np.float32).reshape(4, 128, L))

        winT = np.empty((DEPTH, 128, 2, 4, 2 * EC), f16)
        convD = np.zeros((DEPTH, 2, 128, NJ, DCONV, 128), f16)
        cb = np.empty((DEPTH, 2, 128, NJ), np.float32)
        wxpT = np.empty((DEPTH, 2, 128, NJ, R2), f16)
        wdtT = np.empty((DEPTH, 2, DTR, NJ, 128), f16)
        bdt = np.empty((DEPTH, 2, 128, NJ), np.float32)
        aexp = np.empty((DEPTH, 2, 128, NJ, N), np.float32)
        dpD = np.zeros((DEPTH, 2, 128, NJ, 128), f16)
        woutT = np.empty((DEPTH, 2, 128, NJ, 4, 128), f16)
        idx = np.arange(128)
        for l in range(DEPTH):
            for d in range(2):
                Wf = W_in[l, d] * norm_w[l][None, :]
                rows = np.concatenate([Wf[e0:e0 + EC, :],
                                       Wf[ED + e0:ED + e0 + EC, :]], 0)
                winT[l, :, d] = rows.T.reshape(4, 128, 2 * EC).transpose(
                    1, 0, 2).astype(f16)
                Wxp = W_xp[l, d]
                for j in range(NJ):
                    ej = slice(e0 + j * 128, e0 + (j + 1) * 128)
                    for k in range(DCONV):
                        convD[l, d, idx, j, k, idx] = \
                            conv_w[l, d, ej, k].astype(f16)
                    cb[l, d, :, j] = conv_b[l, d, ej]
                    wxpT[l, d, :, j, :] = Wxp[:, ej].T.astype(f16)
                    wdtT[l, d, :, j, :] = W_dt[l, d][ej, :].T.astype(f16)
                    bdt[l, d, :, j] = b_dt[l, d, ej]
                    aexp[l, d, :, j, :] = A[l, d, ej, :]
                    dpD[l, d, idx, j, idx] = Dparam[l, d, ej].astype(f16)
                    for gg in range(4):
                        woutT[l, d, :, j, gg, :] = \
                            W_out[l, d][gg * 128:(gg + 1) * 128, ej].T \
                            .astype(f16)
        m["winT"] = winT
        m["convD"] = convD
        m["cb"] = cb
        m["wxpT"] = wxpT
        m["wdtT"] = wdtT
        m["bdt"] = bdt
        m["aexp"] = aexp
        m["dpD"] = dpD
        m["woutT"] = woutT

        Ev = np.zeros((VSP, D), np.float32)
        Ev[:VS] = E[r * VS:(r + 1) * VS] * out_norm_w[None, :]
        m["eT"] = np.ascontiguousarray(
            Ev.T.reshape(4, 128, EGRP, ETIL * 128).transpose(
                2, 0, 1, 3)).astype(f16)
        m["ones1"] = np.ones((1, 128), f16)
        m["zero3"] = np.zeros((128, 3), f16)
        m["onesc"] = np.ones((128, 1), f16)
        m["ident"] = np.eye(128).astype(f16)
        in_maps.append(m)
    return in_maps, struct_ok


def kernel(**inputs):
    in_maps, struct_ok = _prep_inputs(inputs)
    key = not struct_ok
    if key not in _BUILT:
        _BUILT[key] = _build(generic_exp=key)
    nc = _BUILT[key]
    res = run_bass_kernel_spmd(nc, in_maps, core_ids=list(range(N_CORES)))
    out = np.empty((B, L, VOCAB), np.float32)
    for c in range(N_CORES):
        g, r = divmod(c, GROUP)
        out[g, :, r * VS:(r + 1) * VS] = \
            res.results[c]["logits"][:VS].T.astype(np.float32)
    return out


if __name__ == "__main__":
    sys.path.insert(0, os.path.dirname(os.path.abspath(__file__)))
    import reference
    ins = {k: np.asarray(v) for k, v in reference.setup_inputs().items()}
    got = kernel(**ins)
    exp = np.asarray(reference.reference(**ins))
    rel = np.abs(got - exp).max() / np.abs(exp).max()
    print("Relative error:", rel)
